# revision 1
# baseline (speedup 1.0000x reference)
"""Distributed GCN (2x GCNConv + MLP head) on 8 Trainium2 NeuronCores.

Raw-bass SPMD kernel (one graph runs on all 8 cores), explicit semaphores.

  - Balanced node partition across 8 cores x NB blocks of 128 nodes so every
    (core, block) has the same compile-time chunk count.
  - Dense matmuls sharded by node; weights replicated; h kept feature-major
    (hT: [256 feats -> 2 x 128-partition tiles, nodes on free dim]).
  - Per GCN layer: hW = h @ Wg (row-major bf16) -> DRAM shard -> AllGather ->
    per-edge indirect-DMA row gather + scatter-add on the PE via matmul with a
    one-hot-times-norm selection matrix S built on the vector engine.
  - Self-loops folded in as ordinary edges with norm = 1/deg.
"""

import contextlib

import numpy as np
import ml_dtypes

import concourse.bass as bass
import concourse.mybir as mybir
from concourse.bass_utils import run_bass_kernel_spmd

FP32 = mybir.dt.float32
BF16 = mybir.dt.bfloat16
FP8 = mybir.dt.float8e4
I32 = mybir.dt.int32
EXDT = BF16  # dtype of the exchanged hW table + gathered messages
SPLIT_CC = 1  # number of independent sub-AllGathers per layer (1 or 2; 2 measured slower)
AF = mybir.ActivationFunctionType
ALU = mybir.AluOpType

C = 8
PB = 128


class Cfg:
    def __init__(self, n, e, d_in, h, d_out, nb):
        self.N = n
        self.E = e
        self.DIN = d_in
        self.H = h
        self.DOUT = d_out
        self.NB = nb
        self.NLOC = nb * PB
        assert self.NLOC * C >= n


FULL = Cfg(50000, 300000, 128, 256, 256, 49)


# ---------------------------------------------------------------- host preproc
def preprocess(cfg, x, edge_index):
    N, NB = cfg.N, cfg.NB
    nblocks = C * NB
    src = np.asarray(edge_index[0], dtype=np.int64)
    dst = np.asarray(edge_index[1], dtype=np.int64)
    indeg = np.bincount(dst, minlength=N).astype(np.int64)
    deg = indeg + 1
    dinv = (1.0 / np.sqrt(deg.astype(np.float64))).astype(np.float32)

    # balanced assignment of nodes to blocks (weight = deg incl self-loop)
    w = deg
    order = np.argsort(-w, kind="stable")
    import heapq

    heap = [(0, b) for b in range(nblocks)]
    heapq.heapify(heap)
    cnt = np.zeros(nblocks, np.int64)
    sumw = np.zeros(nblocks, np.int64)
    blk_of = np.empty(N, np.int32)
    slot_of = np.empty(N, np.int32)
    for v in order:
        while True:
            sw, b = heapq.heappop(heap)
            if cnt[b] < PB:
                break
        blk_of[v] = b
        slot_of[v] = cnt[b]
        cnt[b] += 1
        sumw[b] += w[v]
        if cnt[b] < PB:
            heapq.heappush(heap, (sumw[b], b))
    maxw = int(sumw.max())
    KC = max(2, int(-(-maxw // PB)))
    cap = KC * PB

    core_of_node = (blk_of // NB).astype(np.int32)
    lblk_of_node = (blk_of % NB).astype(np.int32)
    new_row = core_of_node.astype(np.int64) * cfg.NLOC + lblk_of_node * PB + slot_of

    e_blk = blk_of[dst]
    all_blk = np.concatenate([e_blk, blk_of])
    all_dslot = np.concatenate([slot_of[dst], slot_of]).astype(np.float32)
    all_srow = np.concatenate([new_row[src], new_row]).astype(np.int64)
    all_norm = np.concatenate([dinv[src] * dinv[dst], dinv * dinv]).astype(np.float32)

    sort_key = all_blk.astype(np.int64) * (C * cfg.NLOC + 1) + all_srow
    so = np.argsort(sort_key, kind="stable")
    all_blk, all_dslot = all_blk[so], all_dslot[so]
    all_srow, all_norm = all_srow[so], all_norm[so]

    starts = np.searchsorted(all_blk, np.arange(nblocks))
    ends = np.searchsorted(all_blk, np.arange(nblocks) + 1)

    NBK = NB * KC
    H0 = (-(-NB // 2)) * PB  # first-half rows (block-aligned)
    H1 = cfg.NLOC - H0

    def remap_rows(g):
        if SPLIT_CC == 1:
            return g
        c_ = g // cfg.NLOC
        i_ = g % cfg.NLOC
        return np.where(i_ < H0, c_ * H0 + i_, C * H0 + c_ * H1 + (i_ - H0))

    esrcT = np.zeros((C, PB, NBK), np.int32)
    edstT = np.zeros((C, PB, NBK), np.float32)
    enormT = np.zeros((C, PB, NBK), np.float32)
    for b in range(nblocks):
        c, lb = b // NB, b % NB
        s, e = starts[b], ends[b]
        n = e - s
        assert n <= cap, f"block {b} has {n} edges > cap {cap}"
        srow = np.full(cap, c * cfg.NLOC, np.int32)
        dslot = np.zeros(cap, np.float32)
        norm = np.zeros(cap, np.float32)
        srow[:n] = remap_rows(all_srow[s:e])
        dslot[:n] = all_dslot[s:e]
        norm[:n] = all_norm[s:e]
        sl = slice(lb * KC, lb * KC + KC)
        esrcT[c, :, sl] = srow.reshape(KC, PB).T
        edstT[c, :, sl] = dslot.reshape(KC, PB).T
        enormT[c, :, sl] = norm.reshape(KC, PB).T

    xT = np.zeros((C, cfg.DIN, cfg.NLOC), np.float32)
    xx = np.asarray(x, np.float32)
    loc = lblk_of_node.astype(np.int64) * PB + slot_of
    for c in range(C):
        m = core_of_node == c
        xT[c][:, loc[m]] = xx[m].T

    return dict(KC=KC, esrcT=esrcT, edstT=edstT, enormT=enormT, xT=xT, new_row=new_row)


def pack_weights(cfg, W1, b1, Wg1, bg1, Wg2, bg2, W2, b2, W3, b3):
    bf = ml_dtypes.bfloat16

    def packk(Wm):
        k = Wm.shape[0] // 128
        return np.concatenate([Wm[i * 128 : (i + 1) * 128] for i in range(k)], axis=1)

    def bias2(bv):
        return np.stack([bv[:128], bv[128:]], axis=1).astype(np.float32)

    return dict(
        W1=np.asarray(W1, np.float32),
        b1p=bias2(np.asarray(b1)),
        wg1p=packk(np.asarray(Wg1)).astype(bf),
        bg1p=bias2(np.asarray(bg1)),
        wg2p=packk(np.asarray(Wg2)).astype(bf),
        bg2p=bias2(np.asarray(bg2)),
        w2p=packk(np.asarray(W2)).astype(bf),
        b2p=bias2(np.asarray(b2)),
        w3p=packk(np.asarray(W3)).astype(bf),
        b3bc=np.tile(np.asarray(b3, np.float32)[None, :], (PB, 1)),
        iota=np.tile(np.arange(PB, dtype=np.float32)[None, :], (PB, 1)),
    )


# ------------------------------------------------------------- op-list program
class Prog:
    ENGS = ("sync", "pe", "dve", "act", "pool")

    def __init__(self):
        self.ops = {e: [] for e in self.ENGS}
        self.tick = {}

    def emit(self, eng, fn, waits=(), inc=None, inc_by=1):
        t = None
        if inc is not None:
            self.tick[inc] = self.tick.get(inc, 0) + inc_by
            t = self.tick[inc]
        self.ops[eng].append((fn, tuple(waits), inc, inc_by))
        return t


def build_graph(cfg, KC, with_cc=True, with_gather=True):
    NB, NLOC, H, DOUT = cfg.NB, cfg.NLOC, cfg.H, cfg.DOUT
    NBK = NB * KC
    NRM = 3 * KC  # msg/S ring depth (in chunks)
    ACHUNK = 512
    NCH_A = -(-NLOC // ACHUNK)

    nc = bass.Bass()

    def dparam(name, shape, dt):
        return nc.declare_dram_parameter(name, shape, dt, isOutput=False)

    p_xT = dparam("xT", [cfg.DIN, NLOC], FP32)
    p_W1 = dparam("W1", [cfg.DIN, H], FP32)
    p_b1p = dparam("b1p", [PB, 2], FP32)
    p_wg1p = dparam("wg1p", [PB, 2 * H], BF16)
    p_bg1p = dparam("bg1p", [PB, 2], FP32)
    p_wg2p = dparam("wg2p", [PB, 2 * H], BF16)
    p_bg2p = dparam("bg2p", [PB, 2], FP32)
    p_w2p = dparam("w2p", [PB, 2 * H], BF16)
    p_b2p = dparam("b2p", [PB, 2], FP32)
    p_w3p = dparam("w3p", [PB, 2 * DOUT], BF16)
    p_b3bc = dparam("b3bc", [PB, DOUT], FP32)
    p_iota = dparam("iota", [PB, PB], FP32)
    p_esrc = dparam("esrcT", [PB, NBK], I32)
    p_edst = dparam("edstT", [PB, NBK], FP32)
    p_enorm = dparam("enormT", [PB, NBK], FP32)
    p_out = nc.declare_dram_parameter("out", [NLOC, DOUT], FP32, isOutput=True)

    hw_shard = [nc.dram_tensor(f"hw{i}_shard", [NLOC, H], EXDT) for i in (1, 2)]
    hw_full = [
        nc.dram_tensor(f"hw{i}_full", [C * NLOC, H], EXDT, addr_space="Shared")
        for i in (1, 2)
    ]

    with contextlib.ExitStack() as ctx:
        sb = lambda name, shape, dt: ctx.enter_context(nc.sbuf_tensor(name, shape, dt))
        pst = lambda name, shape: ctx.enter_context(nc.psum_tensor(name, shape, FP32))

        xT_sb = sb("xT_sb", [cfg.DIN, NLOC], FP32)
        W1_sb = sb("W1_sb", [cfg.DIN, H], FP32)
        b1p_sb = sb("b1p_sb", [PB, 2], FP32)
        wg1_sb = sb("wg1_sb", [PB, 2 * H], BF16)
        bg1p_sb = sb("bg1p_sb", [PB, 2], FP32)
        wg2_sb = sb("wg2_sb", [PB, 2 * H], BF16)
        bg2p_sb = sb("bg2p_sb", [PB, 2], FP32)
        w2_sb = sb("w2_sb", [PB, 2 * H], BF16)
        b2p_sb = sb("b2p_sb", [PB, 2], FP32)
        w3_sb = sb("w3_sb", [PB, 2 * DOUT], BF16)
        b3bc_sb = sb("b3bc_sb", [PB, DOUT], FP32)
        iota_sb = sb("iota_sb", [PB, PB], FP32)
        esrc_sb = sb("esrc_sb", [PB, NBK], I32)
        edst_sb = sb("edst_sb", [PB, NBK], FP32)
        enorm_sb = sb("enorm_sb", [PB, NBK], FP32)

        hT = {
            1: [sb(f"h1T_{j}", [PB, NLOC], BF16) for j in range(2)],
            2: [sb(f"h2T_{j}", [PB, NLOC], BF16) for j in range(2)],
            3: [sb(f"h3T_{j}", [PB, NLOC], BF16) for j in range(2)],
            4: [sb(f"h4T_{j}", [PB, NLOC], BF16) for j in range(2)],
        }
        msg_ring = sb("msg_ring", [PB, NRM * H], EXDT)
        s_ring = sb("s_ring", [PB, NRM * PB], BF16)
        bev_ring = sb("bev_ring", [PB, 4 * H], EXDT)
        lg_ring = sb("lg_ring", [PB, 2 * DOUT], FP32)
        ex_ring = sb("ex_ring", [PB, 2 * DOUT], FP32)
        ot_ring = sb("ot_ring", [PB, 2 * DOUT], FP32)
        sm_cols = sb("sm_cols", [PB, 8], FP32)

        psA = [pst(f"psA_{i}", [PB, 512]) for i in range(2)]
        psB = [pst(f"psB_{i}", [PB, H]) for i in range(2)]
        psD = [pst(f"psD_{i}", [PB, PB]) for i in range(4)]

        P = Prog()

        # ------------- const loads
        loads = [
            (xT_sb, p_xT), (W1_sb, p_W1), (b1p_sb, p_b1p), (wg1_sb, p_wg1p),
            (bg1p_sb, p_bg1p), (wg2_sb, p_wg2p), (bg2p_sb, p_bg2p),
            (w2_sb, p_w2p), (b2p_sb, p_b2p), (w3_sb, p_w3p), (b3bc_sb, p_b3bc),
            (iota_sb, p_iota), (esrc_sb, p_esrc), (edst_sb, p_edst),
            (enorm_sb, p_enorm),
        ]
        for t, pp in loads:
            P.emit("sync", (lambda t=t, pp=pp: lambda e: e.dma_start(t[:], pp[:]))(),
                   inc="c16", inc_by=16)
        C16_ALL = P.tick["c16"]
        for eng in ("pe", "dve", "act", "pool"):
            P.emit(eng, lambda e: None, waits=[("c16", C16_ALL)])

        # displacement histories (flat, per resource ring)
        psA_hist = []   # ("act1", tick) per alloc
        psB_hist = []   # ("dve1", tick) per alloc
        psD_hist = []   # ("dve1", tick) per alloc
        mm_last = {}    # global chunk idx -> pe tick of its last matmul
        bev_hist = []   # hw16 thresh per bev alloc
        ot_hist = []    # hw16 thresh per ot alloc
        lgex_hist = []  # ("dve1"/"act1", tick) per lg/ex alloc (freed by mul)
        evD = {}        # (li, nb, j) -> dve tick
        cc_t = {}

        def fold(waits):
            m = {}
            for s, v in waits:
                if v is None:
                    continue
                m[s] = max(m.get(s, 0), v)
            return list(m.items())

        # ------------- stage A
        a_evt = {}
        for j in range(2):
            for si in range(NCH_A):
                s = si * ACHUNK
                wd = min(ACHUNK, NLOC - s)
                ai = len(psA_hist)
                ps = psA[ai % 2]
                waits = [psA_hist[ai - 2]] if ai >= 2 else []
                P.emit("pe",
                       (lambda ps=ps, j=j, s=s, wd=wd: lambda e: e.matmul(
                           ps[:, :wd], lhsT=W1_sb[:, j * PB : (j + 1) * PB],
                           rhs=xT_sb[:, s : s + wd], start=True, stop=True))(),
                       waits=fold(waits), inc="pe1")
                mmt = P.tick["pe1"]
                t = P.emit("act",
                           (lambda ps=ps, j=j, s=s, wd=wd: lambda e: e.activation(
                               hT[1][j][:, s : s + wd], ps[:, :wd], AF.Relu,
                               bias=b1p_sb[:, j : j + 1]))(),
                           waits=[("pe1", mmt)], inc="act1")
                a_evt[(j, si)] = t
                psA_hist.append(("act1", t))

        # ------------- stage B (hW matmul + write + AllGather)
        def stage_B(li, hin_key, wg_sb):
            hin = hT[hin_key]
            for nb in range(NB):
                bi = len(psB_hist)
                ps = psB[bi % 2]
                if li == 1:
                    ready = ("act1", max(a_evt[(0, (nb * PB) // ACHUNK)],
                                         a_evt[(1, (nb * PB) // ACHUNK)]))
                else:
                    ready = ("dve1", max(evD[(1, nb, 0)], evD[(1, nb, 1)]))
                waits = [ready]
                if bi >= 2:
                    waits.append(psB_hist[bi - 2])
                P.emit("pe",
                       (lambda ps=ps, hin=hin, nb=nb, wg_sb=wg_sb: lambda e: e.matmul(
                           ps[:], lhsT=hin[0][:, nb * PB : (nb + 1) * PB],
                           rhs=wg_sb[:, 0:H], start=True, stop=False))(),
                       waits=fold(waits), inc="pe1")
                P.emit("pe",
                       (lambda ps=ps, hin=hin, nb=nb, wg_sb=wg_sb: lambda e: e.matmul(
                           ps[:], lhsT=hin[1][:, nb * PB : (nb + 1) * PB],
                           rhs=wg_sb[:, H : 2 * H], start=False, stop=True))(),
                       inc="pe1")
                mmt = P.tick["pe1"]
                # evac psum -> bf16 ring (DVE)
                bevi = len(bev_hist)
                bslot = bevi % 4
                ew = [("pe1", mmt)]
                if bevi >= 4:
                    ew.append(bev_hist[bevi - 4])
                et = P.emit("dve",
                            (lambda ps=ps, bslot=bslot: lambda e: e.tensor_copy(
                                bev_ring[:, bslot * H : (bslot + 1) * H], ps[:]))(),
                            waits=fold(ew), inc="dve1")
                psB_hist.append(("dve1", et))
                P.emit("sync",
                       (lambda li=li, nb=nb, bslot=bslot: lambda e: e.dma_start(
                           hw_shard[li - 1][nb * PB : (nb + 1) * PB, :],
                           bev_ring[:, bslot * H : (bslot + 1) * H]))(),
                       waits=[("dve1", et)], inc=f"bw{bslot}", inc_by=16)
                bev_hist.append((f"bw{bslot}", P.tick[f"bw{bslot}"]))
            ccw = [(f"bw{s}", P.tick.get(f"bw{s}", 0)) for s in range(4)
                   if P.tick.get(f"bw{s}", 0) > 0]
            if with_cc:
                H0b = (-(-NB // 2)) * PB
                if SPLIT_CC == 1:
                    parts = [(0, NLOC, 0)]
                else:
                    parts = [(0, H0b, 0), (H0b, NLOC, C * H0b)]
                for (r0, r1, o0) in parts:
                    P.emit("pool",
                           (lambda li=li, r0=r0, r1=r1, o0=o0: lambda e:
                            e.collective_compute(
                                "AllGather", ALU.bypass,
                                replica_groups=[list(range(C))],
                                ins=[hw_shard[li - 1][r0:r1, :].opt()],
                                outs=[hw_full[li - 1][o0 : o0 + C * (r1 - r0), :].opt()]))(),
                           waits=ccw, inc="cc1")
                cc_t[li] = P.tick["cc1"]
            else:
                cc_t[li] = 0

        # ------------- stage D (message passing)
        g16_of = {}
        s_dve = {}

        def stage_D(li, hout_key, bgp_sb):
            hout = hT[hout_key]

            def emit_evac(nb):
                for j in range(2):
                    di = 2 * ((li - 1) * NB + nb) + j
                    ps = psD[di % 4]
                    t = P.emit(
                        "dve",
                        (lambda ps=ps, j=j, nb=nb, hout=hout, bgp_sb=bgp_sb: lambda e:
                         e.tensor_scalar(
                             out=hout[j][:, nb * PB : (nb + 1) * PB], in0=ps[:],
                             scalar1=bgp_sb[:, j : j + 1], scalar2=0.0,
                             op0=ALU.add, op1=ALU.max))(),
                        waits=[("pe1", mm_last[(li - 1) * NBK + nb * KC + KC - 1])],
                        inc="dve1")
                    evD[(li, nb, j)] = t
                    psD_hist[di] = ("dve1", t)

            for nb in range(NB):
                for k in range(KC):
                    ci = nb * KC + k
                    gi = (li - 1) * NBK + ci
                    r = gi % NRM
                    gw = [("cc1", cc_t[li])] if with_cc else []
                    if gi >= NRM:
                        gw.append(("pe1", mm_last[gi - NRM]))
                    if with_gather:
                        P.emit("pool",
                               (lambda li=li, ci=ci, r=r: lambda e: e.indirect_dma_start(
                                   out=msg_ring[:, r * H : (r + 1) * H],
                                   out_offset=None,
                                   in_=hw_full[li - 1][:, :],
                                   in_offset=bass.IndirectOffsetOnAxis(
                                       ap=esrc_sb[:, ci : ci + 1], axis=0)))(),
                               waits=fold(gw), inc=f"gs{r}", inc_by=16)
                        g16_of[gi] = (f"gs{r}", P.tick[f"gs{r}"])
                    else:
                        g16_of[gi] = None
                    sw = []
                    if gi >= NRM:
                        sw.append(("pe1", mm_last[gi - NRM]))
                    s_dve[gi] = P.emit(
                        "dve",
                        (lambda ci=ci, r=r: lambda e: e.tensor_scalar(
                            out=s_ring[:, r * PB : (r + 1) * PB], in0=iota_sb[:],
                            scalar1=edst_sb[:, ci : ci + 1],
                            scalar2=enorm_sb[:, ci : ci + 1],
                            op0=ALU.is_equal, op1=ALU.mult))(),
                        waits=fold(sw), inc="dve1")
                if nb >= 1:
                    emit_evac(nb - 1)
                for j in range(2):
                    di = 2 * ((li - 1) * NB + nb) + j
                    ps = psD[di % 4]
                    for k in range(KC):
                        ci = nb * KC + k
                        gi = (li - 1) * NBK + ci
                        r = gi % NRM
                        waits = ([g16_of[gi]] if g16_of[gi] is not None else []) + [("dve1", s_dve[gi])]
                        if k == 0 and di >= 4 and psD_hist[di - 4] is not None:
                            waits.append(psD_hist[di - 4])
                        P.emit("pe",
                               (lambda ps=ps, j=j, r=r, k=k: lambda e: e.matmul(
                                   ps[:],
                                   lhsT=msg_ring[:, r * H + j * PB : r * H + (j + 1) * PB],
                                   rhs=s_ring[:, r * PB : (r + 1) * PB],
                                   start=(k == 0), stop=(k == KC - 1)))(),
                               waits=fold(waits), inc="pe1")
                        if j == 1:
                            mm_last[gi] = P.tick["pe1"]
            emit_evac(NB - 1)

        # pre-size psD_hist
        psD_hist.extend([None] * (4 * NB))  # will be replaced; sized for 2 layers below
        psD_hist.extend([None] * (4 * NB))

        stage_B(1, 1, wg1_sb)
        stage_D(1, 2, bg1p_sb)
        stage_B(2, 2, wg2_sb)
        stage_D(2, 3, bg2p_sb)

        # ------------- stage G
        evG = {}
        for j in range(2):
            for si in range(NCH_A):
                s = si * ACHUNK
                wd = min(ACHUNK, NLOC - s)
                ai = len(psA_hist)
                ps = psA[ai % 2]
                b0, b1_ = s // PB, (s + wd - 1) // PB
                ready = max(evD[(2, b, jj)] for b in range(b0, b1_ + 1) for jj in range(2))
                for k in range(2):
                    waits = []
                    if k == 0:
                        waits = [("dve1", ready), psA_hist[ai - 2]]
                    P.emit("pe",
                           (lambda ps=ps, j=j, s=s, wd=wd, k=k: lambda e: e.matmul(
                               ps[:, :wd],
                               lhsT=w2_sb[:, k * H + j * PB : k * H + (j + 1) * PB],
                               rhs=hT[3][k][:, s : s + wd],
                               start=(k == 0), stop=(k == 1)))(),
                           waits=fold(waits), inc="pe1")
                mmt = P.tick["pe1"]
                t = P.emit("act",
                           (lambda ps=ps, j=j, s=s, wd=wd: lambda e: e.activation(
                               hT[4][j][:, s : s + wd], ps[:, :wd], AF.Relu,
                               bias=b2p_sb[:, j : j + 1]))(),
                           waits=[("pe1", mmt)], inc="act1")
                evG[(j, si)] = t
                psA_hist.append(("act1", t))

        # ------------- stage H
        h_exp = {}
        h_mul = {}
        for nb in range(NB):
            bi = len(psB_hist)
            ps = psB[bi % 2]
            ready = ("act1", max(evG[(0, (nb * PB) // ACHUNK)],
                                 evG[(1, (nb * PB) // ACHUNK)]))
            for k in range(2):
                waits = []
                if k == 0:
                    waits = [ready, psB_hist[bi - 2]]
                P.emit("pe",
                       (lambda ps=ps, nb=nb, k=k: lambda e: e.matmul(
                           ps[:], lhsT=hT[4][k][:, nb * PB : (nb + 1) * PB],
                           rhs=w3_sb[:, k * DOUT : (k + 1) * DOUT],
                           start=(k == 0), stop=(k == 1)))(),
                       waits=fold(waits), inc="pe1")
            mmt = P.tick["pe1"]
            ls = (nb % 2) * DOUT
            # lg slot WAR: exp of nb-2 read it; ex slot WAR: mul of nb-2
            addw = [("pe1", mmt)]
            if nb >= 2:
                addw.append(("act1", h_exp[nb - 2]))
            t_add = P.emit("dve",
                           (lambda ps=ps, ls=ls: lambda e: e.tensor_add(
                               lg_ring[:, ls : ls + DOUT], ps[:], b3bc_sb[:]))(),
                           waits=fold(addw), inc="dve1")
            psB_hist.append(("dve1", t_add))
            cs = (nb % 4) * 2
            expw = [("dve1", t_add)]
            if nb >= 2:
                expw.append(("dve1", h_mul[nb - 2]))
            h_exp[nb] = P.emit(
                "act",
                (lambda ls=ls, cs=cs: lambda e: e.activation(
                    ex_ring[:, ls : ls + DOUT], lg_ring[:, ls : ls + DOUT], AF.Exp,
                    accum_out=sm_cols[:, cs : cs + 1]))(),
                waits=fold(expw), inc="act1")
            t_rec = P.emit("dve",
                           (lambda cs=cs: lambda e: e.reciprocal(
                               sm_cols[:, cs + 1 : cs + 2], sm_cols[:, cs : cs + 1]))(),
                           waits=[("act1", h_exp[nb])], inc="dve1")
            mulw = [("dve1", t_rec)]
            if nb >= 2:
                mulw.append(ot_hist[nb - 2])
            h_mul[nb] = P.emit(
                "dve",
                (lambda ls=ls, cs=cs: lambda e: e.tensor_scalar_mul(
                    ot_ring[:, ls : ls + DOUT], ex_ring[:, ls : ls + DOUT],
                    sm_cols[:, cs + 1 : cs + 2]))(),
                waits=fold(mulw), inc="dve1")
            oslot = nb % 2
            P.emit("sync",
                   (lambda nb=nb, ls=ls: lambda e: e.dma_start(
                       p_out[nb * PB : (nb + 1) * PB, :], ot_ring[:, ls : ls + DOUT]))(),
                   waits=[("dve1", h_mul[nb])], inc=f"ow{oslot}", inc_by=16)
            ot_hist.append((f"ow{oslot}", P.tick[f"ow{oslot}"]))

        finw = [(s, P.tick[s]) for s in P.tick
                if s.startswith("bw") or s.startswith("ow")]
        P.emit("sync", lambda e: None, waits=finw)
        gfin = [(s, P.tick[s]) for s in P.tick if s.startswith("gs")]
        if gfin:
            P.emit("pool", lambda e: None, waits=gfin)

        # ------------- emit per-engine programs
        sems = {}
        semnames = ["c16", "cc1", "pe1", "dve1", "act1"]
        semnames += [f"gs{i}" for i in range(NRM)]
        semnames += [f"bw{i}" for i in range(4)] + ["ow0", "ow1"]
        for s in semnames:
            sems[s] = ctx.enter_context(nc.semaphore(s))

        with nc.Block() as block:

            def mk_body(eng_name):
                def body(e):
                    last = {}
                    for fn, waits, inc, inc_by in P.ops[eng_name]:
                        for s, v in waits:
                            if v is None or last.get(s, 0) >= v:
                                continue
                            e.wait_ge(sems[s], v)
                            last[s] = v
                        ins = fn(e)
                        if inc is not None and ins is not None:
                            ins.then_inc(sems[inc], inc_by)
                return body

            block.sync(mk_body("sync"))
            block.tensor(mk_body("pe"))
            block.vector(mk_body("dve"))
            block.scalar(mk_body("act"))
            block.gpsimd(mk_body("pool"))

    return nc


# ---------------------------------------------------------------- entry point
def build_in_maps(cfg, prep, wts):
    in_maps = []
    for c in range(C):
        in_maps.append(dict(
            xT=prep["xT"][c], W1=wts["W1"], b1p=wts["b1p"], wg1p=wts["wg1p"],
            bg1p=wts["bg1p"], wg2p=wts["wg2p"], bg2p=wts["bg2p"], w2p=wts["w2p"],
            b2p=wts["b2p"], w3p=wts["w3p"], b3bc=wts["b3bc"], iota=wts["iota"],
            esrcT=prep["esrcT"][c], edstT=prep["edstT"][c], enormT=prep["enormT"][c],
        ))
    return in_maps


def run(cfg, inputs, trace=False):
    prep = preprocess(cfg, inputs["x"], inputs["edge_index"])
    wts = pack_weights(
        cfg,
        inputs["W1"], inputs["b1"], inputs["Wg1"], inputs["bg1"],
        inputs["Wg2"], inputs["bg2"], inputs["W2"], inputs["b2"],
        inputs["W3"], inputs["b3"],
    )
    nc = build_graph(cfg, prep["KC"])
    in_maps = build_in_maps(cfg, prep, wts)
    res = run_bass_kernel_spmd(nc, in_maps, list(range(C)), trace=trace)
    shards = np.concatenate([np.asarray(res.results[c]["out"]) for c in range(C)], axis=0)
    out = shards[prep["new_row"]]
    return np.ascontiguousarray(out, dtype=np.float32), res


def kernel(**inputs):
    out, _ = run(FULL, inputs, trace=False)
    return out

